# revision 53
# baseline (speedup 1.0000x reference)
"""Bass/Trainium2 kernel for nn_Graph_Layer (gnn_message_passing).

Reference math (N=8192, D=512):
    G0[i,j] = ||s_i - s_j + eps||_2   (pairwise distances, Gram trick)
    G = 1 - G0 / rowmax(G0)
    out = (G @ x) @ W

Decomposition (row-shard over 8 cores, 1024 rows each):
    sqd[i,j] = ri[i] + cj[j] - 2*gram[i,j]        (ri, cj host-precomputed)
    G0 = sqrt(sqd + CLAMP)                         (CLAMP covers tf32 noise on diag)
    rowmax[i] = max_j G0[i,j]
    (G @ x)[i,:] = colsum_x - Y0[i,:]/rowmax[i],   Y0 = G0 @ x
    out[i,:]  = w2 - (Y0 @ W)[i,:]/rowmax[i],      w2 = colsum_x @ W (host)

On device the distance strip is computed TRANSPOSED (sqd^T[j,i]) so G0 tiles
come out with j (the contraction dim of Y0) on partitions. The per-i "ri" term
is added by the Vector engine from a host-precomputed broadcast tile (riB)
instead of a 1-row PE matmul: a 1-row matmul costs the same PE cycles as a
full one (cost = output free size), and its LDWEIGHTS stalled the PE pipeline
every iteration, dropping the p-state.

Y0 is accumulated transposed (Y0T[c,i], stationary = x c-slices, moving = G0)
so no transposes are needed before the W GEMM; the GEMM output lands [i, n]
with i on partitions, where the -1/rowmax scale is a per-partition ACT scale
and w2 is a DVE add of a host broadcast tile.

Each core sees its own np.roll'ed copy of the inputs so local rows are always
[0,1024): a single uniform SPMD program runs on all 8 cores. The gram and Y
matmuls run in bf16 (inputs host-rounded; ri/cj/colsum/w2 derived from the
ROUNDED values so the device computes exact quantities of the rounded data and
the bf16 error enters only through small terms); Y uses centered c0 = G0 - 32
to keep bf16 rounding ~30x smaller, with the 32*colsum component restored via
w2 in the tail. The final W GEMM stays float32r.
"""

import numpy as np
import ml_dtypes
from contextlib import ExitStack

import concourse.bass as bass
from concourse import bacc
import concourse.tile as tile
from concourse import mybir
from concourse.bass_utils import run_bass_kernel_spmd
from concourse.masks import make_identity

N, D, NOUT = 8192, 512, 512
M = 8                 # cores
R = N // M            # 1024 local rows per core
EPS = 1e-6
CLAMP = 0.3           # covers tf32 rounding noise on the diagonal; ~1e-4 rel effect off-diag
F32 = mybir.dt.float32
F32R = mybir.dt.float32r
BF16 = mybir.dt.bfloat16

KT = D // 128         # 4 contraction sub-tiles
NJT = N // 128        # 64 j tiles
IB = 512              # i block (free dim of the gram matmuls)
NIB = R // IB         # 2
NSUB = IB // 128      # 4 sub-tiles of 128 rows per i block

CH = 512              # S^T DMA chunk width (columns); chunk c covers j_tiles 4c..4c+3
NCH = N // CH


def build_kernel(ctx, tc, out_d, x_d, s_d, cj_d, rib_d, w_d, w2b_d):
    nc = tc.nc

    singles = ctx.enter_context(tc.tile_pool(name="singles", bufs=1))
    xt_pool = ctx.enter_context(tc.tile_pool(name="xt", bufs=4))
    g0_pool = ctx.enter_context(tc.tile_pool(name="g0", bufs=3))
    c0_pool = ctx.enter_context(tc.tile_pool(name="c0", bufs=3))
    sqd_pool = ctx.enter_context(tc.tile_pool(name="sqd", bufs=3))
    y0t_pool = ctx.enter_context(tc.tile_pool(name="y0t", bufs=4))
    osb_pool = ctx.enter_context(tc.tile_pool(name="osb", bufs=4))
    sm_pool = ctx.enter_context(tc.tile_pool(name="sm", bufs=4))
    macc_pool = ctx.enter_context(tc.tile_pool(name="macc", bufs=2))
    ps_tr = ctx.enter_context(tc.tile_pool(name="ps_tr", bufs=1, space="PSUM"))
    ps_g = ctx.enter_context(tc.tile_pool(name="ps_g", bufs=2, space="PSUM"))
    ps_y = ctx.enter_context(tc.tile_pool(name="ps_y", bufs=1, space="PSUM"))
    ps_o = ctx.enter_context(tc.tile_pool(name="ps_o", bufs=1, space="PSUM"))

    # --- persistent SBUF tensors ---
    # S^T lives in one tile PER (chunk, k-slice): dependency tracking is
    # tile-granular, so a single big tile makes every gram LDWEIGHTS emitted
    # after a chunk-DMA burst wait for ALL outstanding chunk writes (~1us
    # false stall per burst through ib0). Per-chunk tiles make deps precise.
    stc = [[singles.tile([128, CH], BF16, name=f"st_{c}_{k}") for k in range(KT)]
           for c in range(NCH)]
    w_sb = singles.tile([128, KT * NOUT], F32R)       # W c-tiles
    cj_sb = singles.tile([128, NJT], F32)             # cj[t*128+p] at [p, t]
    rib_sb = singles.tile([128, R], F32)              # ri/2 broadcast across partitions
    w2b_sb = singles.tile([128, NOUT], F32)           # w2 broadcast across partitions
    ident = singles.tile([128, 128], F32)

    def load_st_chunk(c):
        for k in range(KT):
            nc.sync.dma_start(
                out=stc[c][k][:],
                in_=s_d[bass.ts(k, 128), c * CH:(c + 1) * CH],
            )

    def st_slice(k, lo, hi):
        """S^T [k-slice, columns lo:hi]; lo:hi never crosses a chunk."""
        c = lo // CH
        return stc[c][k][:, lo - c * CH: hi - c * CH]

    # chunk 0 first: the first gram matmuls are gated on it; riB isn't needed
    # until the first DVE sub, cj until the first ACT, w2B until the tail
    load_st_chunk(0)
    nc.sync.dma_start(out=rib_sb[:], in_=rib_d)
    nc.sync.dma_start(out=cj_sb[:], in_=cj_d)
    nc.sync.dma_start(out=w2b_sb[:], in_=w2b_d)
    make_identity(nc, ident[:])  # only needed at the i-block tails

    def emit_out(ib, s, y0ts, ninv_q):
        """GEMM + scale + w2 + store for one 128-row output sub-tile."""
        ninv, q32 = ninv_q
        pso = ps_o.tile([128, NOUT], F32, tag="o", name=f"pso{ib}_{s}")
        for c in range(KT):
            nc.tensor.matmul(
                pso[:],
                y0ts[c][:, bass.ts(s, 128)],
                w_sb[:, c * NOUT:(c + 1) * NOUT],
                start=(c == 0),
                stop=(c == KT - 1),
            )
        acted = osb_pool.tile([128, NOUT], F32, tag="osb", name=f"acted{ib}_{s}")
        nc.scalar.activation(
            out=acted[:], in_=pso[:],
            func=mybir.ActivationFunctionType.Copy, scale=ninv[:],
        )
        # out = q*w2 + acted  (= w2 - (Yc@W + 32*w2)/rowmax)
        osb = osb_pool.tile([128, NOUT], F32, tag="osb", name=f"osb{ib}_{s}")
        nc.vector.scalar_tensor_tensor(
            out=osb[:], in0=w2b_sb[:], scalar=q32[:], in1=acted[:],
            op0=mybir.AluOpType.mult, op1=mybir.AluOpType.add,
        )
        nc.sync.dma_start(out=out_d[bass.ts(ib * NSUB + s, 128), :], in_=osb[:])

    # --- main: per i-block: gram strip -> G0 -> Y0T accum -> normalize -> GEMM ---
    deferred = []  # output-side tail closures from the previous i-block
    dma_q = []     # pending (chunk, k) background loads, one issued per jt
    for ib in range(NIB):
        icol0 = ib * IB  # local column offset into S^T / riB
        psy = [ps_y.tile([128, IB], F32, tag=f"y{c}", name=f"psy{c}")
               for c in range(KT)]
        macc = macc_pool.tile([128, IB], F32, tag="macc")
        hist = []

        for jt in range(NJT):
            # interleave the previous i-block's output tail into this block's
            # stream so its dependency latency hides under main-loop compute
            if deferred and jt >= 2 and jt % 2 == 0:
                deferred.pop(0)()
            xt = xt_pool.tile([128, D], BF16, tag="xt")
            nc.sync.dma_start(out=xt[:], in_=x_d[bass.ts(jt, 128), :])

            if ib == 0:
                if jt == 0:
                    load_st_chunk(1)
                    load_st_chunk(2)
                elif jt % 4 == 0 and jt // 4 + 2 < NCH:
                    load_st_chunk(jt // 4 + 2)
                if jt == 32:
                    for kt in range(KT):
                        nc.sync.dma_start(
                            out=w_sb[:, kt * NOUT:(kt + 1) * NOUT],
                            in_=w_d[bass.ts(kt, 128), :].bitcast(F32R),
                        )

            psg = ps_g.tile([128, IB], F32, tag="g")
            for k in range(KT):
                nc.tensor.matmul(
                    psg[:],
                    st_slice(k, jt * 128, jt * 128 + 128),
                    st_slice(k, icol0, icol0 + IB),
                    start=(k == 0),
                    stop=(k == KT - 1),
                )
            # sqd = gram - ri/2  (broadcast tile; per-i term of the expansion);
            # PSUM -> SBUF, freeing the psg bank for the next gram group
            sqd = sqd_pool.tile([128, IB], F32, tag="sqd")
            nc.vector.tensor_sub(sqd[:], psg[:], rib_sb[:, icol0:icol0 + IB])

            # G0^T tile = sqrt(-2*sqd + cj[j])   (cj includes +CLAMP)
            g0 = g0_pool.tile([128, IB], F32, tag="g0")
            nc.scalar.activation(
                out=g0[:], in_=sqd[:],
                func=mybir.ActivationFunctionType.Sqrt,
                bias=cj_sb[:, jt:jt + 1], scale=-2.0,
            )
            # centered copy c0 = G0 - 32 in bf16 for the Y matmuls: G0 values
            # cluster near sqrt(2D)=32, so centering keeps the bf16 rounding
            # error ~30x smaller; the 32*colsum component is restored exactly
            # in the tail via w2 (host-computed from the same rounded x).
            # Must stay on the Scalar engine: GpSimd runs this op ~10x slower
            # (microcode overhead) and tanks the whole kernel.
            c0 = c0_pool.tile([128, IB], BF16, tag="c0")
            nc.scalar.activation(
                out=c0[:], in_=g0[:],
                func=mybir.ActivationFunctionType.Copy, bias=-32.0,
            )

            # software pipeline: the Y0T matmuls AND the rowmax update run TWO
            # steps behind the gram, giving the gram->sub->sqrt chain a full
            # extra iteration of slack before Y consumes g0. Emitting sub
            # before max also keeps the psg PSUM recycle off the ACT chain.
            if jt >= 2:
                pg0, pc0, pxt = hist[jt - 2]
                if jt == 2:
                    nc.vector.tensor_copy(out=macc[:], in_=pg0[:])
                else:
                    nc.vector.tensor_max(macc[:], macc[:], pg0[:])
                for c in range(KT):
                    nc.tensor.matmul(
                        psy[c][:], pxt[:, bass.ts(c, 128)], pc0[:],
                        start=(jt == 2), stop=False,
                    )
            hist.append((g0, c0, xt))

        for tail_jt in (NJT - 2, NJT - 1):
            pg0, pc0, pxt = hist[tail_jt]
            nc.vector.tensor_max(macc[:], macc[:], pg0[:])
            for c in range(KT):
                nc.tensor.matmul(
                    psy[c][:], pxt[:, bass.ts(c, 128)], pc0[:],
                    start=False, stop=(tail_jt == NJT - 1),
                )

        # tail, part 1: rowmax -> -1/rowmax per i sub-tile
        ninvs = []
        for s in range(NSUB):
            pst = ps_tr.tile([128, 128], F32, tag="tr")
            nc.tensor.transpose(pst[:], macc[:, bass.ts(s, 128)], ident[:])
            rm = sm_pool.tile([128, 1], F32, tag="rm")
            nc.vector.tensor_reduce(
                out=rm[:], in_=pst[:], axis=mybir.AxisListType.X,
                op=mybir.AluOpType.max,
            )
            nrm = sm_pool.tile([128, 1], F32, tag="nrm")
            nc.vector.tensor_scalar_mul(nrm[:], rm[:], -1.0)
            ninv = sm_pool.tile([128, 1], F32, tag="ninv", name=f"ninv{s}")
            nc.vector.reciprocal(ninv[:], nrm[:])  # -1/rowmax
            # q = 1 - 32/rowmax: scales w2 to restore the centered-away
            # 32*colsum component of Y0 (= G0@x with G0 = c0 + 32)
            q32 = sm_pool.tile([128, 1], F32, tag="q32", name=f"q32{s}")
            nc.vector.tensor_scalar(
                out=q32[:], in0=ninv[:], scalar1=32.0, scalar2=1.0,
                op0=mybir.AluOpType.mult, op1=mybir.AluOpType.add,
            )
            ninvs.append((ninv, q32))

        # tail, part 2: Y0T out of PSUM (frees the psy banks for the next
        # i-block); the GEMM/scale/store closures are deferred into the next
        # i-block's instruction stream (emitted immediately on the last block)
        y0ts = []
        for c in range(KT):
            y0t = y0t_pool.tile([128, IB], F32R, tag="y0t", name=f"y0t{ib}_{c}")
            if c % 2 == 0:
                nc.vector.tensor_copy(out=y0t[:], in_=psy[c][:])
            else:
                nc.scalar.copy(out=y0t[:], in_=psy[c][:])
            y0ts.append(y0t)

        deferred = [
            (lambda ib=ib, s=s, y0ts=y0ts, ninv=ninvs[s]: emit_out(ib, s, y0ts, ninv))
            for s in range(NSUB)
        ]
        if ib == NIB - 1:
            for fn in deferred:
                fn()


_NC_CACHE = {}


def _build_nc():
    if "nc" in _NC_CACHE:
        return _NC_CACHE["nc"]
    nc = bacc.Bacc("TRN2", target_bir_lowering=False, debug=False, num_devices=M)
    x_d = nc.dram_tensor("x", [N, D], BF16, kind="ExternalInput").ap()
    s_d = nc.dram_tensor("simT", [D, N], BF16, kind="ExternalInput").ap()
    cj_d = nc.dram_tensor("cj", [128, NJT], F32, kind="ExternalInput").ap()
    rib_d = nc.dram_tensor("riB", [128, R], F32, kind="ExternalInput").ap()
    w_d = nc.dram_tensor("w", [D, NOUT], F32, kind="ExternalInput").ap()
    w2b_d = nc.dram_tensor("w2B", [128, NOUT], F32, kind="ExternalInput").ap()
    out_d = nc.dram_tensor("out", [R, NOUT], F32, kind="ExternalOutput").ap()
    with tile.TileContext(nc) as tc, ExitStack() as ctx:
        build_kernel(ctx, tc, out_d, x_d, s_d, cj_d, rib_d, w_d, w2b_d)
    nc.compile()
    _NC_CACHE["nc"] = nc
    return nc


def make_in_maps(x, sim_feat, weight):
    w = np.ascontiguousarray(weight, dtype=np.float32)
    # the gram matmuls run in bf16: round sim on the host and derive ri/cj
    # from the ROUNDED values so the device computes exact distances of the
    # rounded vectors (error = distance perturbation only, ~1e-3 relative)
    sim_bf = np.asarray(sim_feat, dtype=np.float32).astype(ml_dtypes.bfloat16)
    # x is also uploaded bf16 (Y stationary). colsum/w2 MUST come from the
    # ROUNDED x: out = w2 - (G0@x_bf@W)/rm cancels the large colsum component
    # exactly only if w2 was built from the same x the device saw; the
    # residual error enters only through G (small), not G0 (large).
    x_bf = np.asarray(x, dtype=np.float32).astype(ml_dtypes.bfloat16)

    sim64 = sim_bf.astype(np.float64)
    sq = (sim64 * sim64).sum(1)
    ss = sim64.sum(1)
    cj_full = (sq - 2.0 * EPS * ss + CLAMP).astype(np.float32)         # [N]
    ri_full = sq + 2.0 * EPS * ss + D * EPS * EPS                      # [N] f64
    colsum = x_bf.astype(np.float64).sum(0)
    w2 = (colsum @ w.astype(np.float64)).astype(np.float32)
    w2B = np.ascontiguousarray(np.broadcast_to(w2, (128, NOUT)))

    in_maps = []
    for c in range(M):
        shift = c * R
        sim_c = np.ascontiguousarray(np.roll(sim_bf, -shift, axis=0).T)
        x_c = np.roll(x_bf, -shift, axis=0)
        cj_c = np.ascontiguousarray(
            np.roll(cj_full, -shift).reshape(NJT, 128).T
        )                                                               # [128, NJT]
        ri_c = (ri_full[shift:shift + R] / 2.0).astype(np.float32)
        rib_c = np.ascontiguousarray(np.broadcast_to(ri_c, (128, R)))
        in_maps.append(
            {"x": x_c, "simT": sim_c, "cj": cj_c, "riB": rib_c,
             "w": w, "w2B": w2B}
        )
    return in_maps


def kernel(x, sim_feat, weight, _trace=False, **kw):
    nc = _build_nc()
    in_maps = make_in_maps(x, sim_feat, weight)
    res = run_bass_kernel_spmd(nc, in_maps, list(range(M)), trace=_trace, **kw)
    out = np.concatenate([res.results[c]["out"] for c in range(M)], axis=0)
    if _trace:
        return out, res
    return out


# revision 55
# speedup vs baseline: 1.1793x; 1.1793x over previous
"""Bass/Trainium2 kernel for nn_Graph_Layer (gnn_message_passing).

Reference math (N=8192, D=512):
    G0[i,j] = ||s_i - s_j + eps||_2   (pairwise distances, Gram trick)
    G = 1 - G0 / rowmax(G0)
    out = (G @ x) @ W

Decomposition (row-shard over 8 cores, 1024 rows each):
    sqd[i,j] = ri[i] + cj[j] - 2*gram[i,j]        (ri, cj host-precomputed)
    G0 = sqrt(sqd + CLAMP)                         (CLAMP covers tf32 noise on diag)
    rowmax[i] = max_j G0[i,j]
    (G @ x)[i,:] = colsum_x - Y0[i,:]/rowmax[i],   Y0 = G0 @ x
    out[i,:]  = w2 - (Y0 @ W)[i,:]/rowmax[i],      w2 = colsum_x @ W (host)

On device the distance strip is computed TRANSPOSED (sqd^T[j,i]) so G0 tiles
come out with j (the contraction dim of Y0) on partitions. The per-i "ri" term
is added by the Vector engine from a host-precomputed broadcast tile (riB)
instead of a 1-row PE matmul: a 1-row matmul costs the same PE cycles as a
full one (cost = output free size), and its LDWEIGHTS stalled the PE pipeline
every iteration, dropping the p-state.

Y0 is accumulated transposed (Y0T[c,i], stationary = x c-slices, moving = G0)
so no transposes are needed before the W GEMM; the GEMM output lands [i, n]
with i on partitions, where the -1/rowmax scale is a per-partition ACT scale
and w2 is a DVE add of a host broadcast tile.

Each core sees its own np.roll'ed copy of the inputs so local rows are always
[0,1024): a single uniform SPMD program runs on all 8 cores. The gram and Y
matmuls run in bf16 (inputs host-rounded; ri/cj/colsum/w2 derived from the
ROUNDED values so the device computes exact quantities of the rounded data and
the bf16 error enters only through small terms); Y uses centered c0 = G0 - 32
to keep bf16 rounding ~30x smaller, with the 32*colsum component restored via
w2 in the tail. The final W GEMM stays float32r.
"""

import numpy as np
import ml_dtypes
from contextlib import ExitStack

import concourse.bass as bass
from concourse import bacc
import concourse.tile as tile
from concourse import mybir
from concourse.bass_utils import run_bass_kernel_spmd
from concourse.masks import make_identity

N, D, NOUT = 8192, 512, 512
M = 8                 # cores
R = N // M            # 1024 local rows per core
EPS = 1e-6
CLAMP = 0.3           # covers tf32 rounding noise on the diagonal; ~1e-4 rel effect off-diag
F32 = mybir.dt.float32
F32R = mybir.dt.float32r
BF16 = mybir.dt.bfloat16

KT = D // 128         # 4 contraction sub-tiles
NJT = N // 128        # 64 j tiles
IB = 512              # i block (free dim of the gram matmuls)
NIB = R // IB         # 2
NSUB = IB // 128      # 4 sub-tiles of 128 rows per i block

CH = 512              # S^T DMA chunk width (columns); chunk c covers j_tiles 4c..4c+3
NCH = N // CH


def build_kernel(ctx, tc, out_d, x_d, s_d, cj_d, rib_d, w_d, w2b_d):
    nc = tc.nc

    singles = ctx.enter_context(tc.tile_pool(name="singles", bufs=1))
    xt_pool = ctx.enter_context(tc.tile_pool(name="xt", bufs=4))
    g0_pool = ctx.enter_context(tc.tile_pool(name="g0", bufs=3))
    c0_pool = ctx.enter_context(tc.tile_pool(name="c0", bufs=3))
    sqd_pool = ctx.enter_context(tc.tile_pool(name="sqd", bufs=3))
    y0t_pool = ctx.enter_context(tc.tile_pool(name="y0t", bufs=4))
    osb_pool = ctx.enter_context(tc.tile_pool(name="osb", bufs=4))
    sm_pool = ctx.enter_context(tc.tile_pool(name="sm", bufs=4))
    macc_pool = ctx.enter_context(tc.tile_pool(name="macc", bufs=2))
    ps_tr = ctx.enter_context(tc.tile_pool(name="ps_tr", bufs=1, space="PSUM"))
    ps_g = ctx.enter_context(tc.tile_pool(name="ps_g", bufs=2, space="PSUM"))
    ps_y = ctx.enter_context(tc.tile_pool(name="ps_y", bufs=1, space="PSUM"))
    ps_o = ctx.enter_context(tc.tile_pool(name="ps_o", bufs=1, space="PSUM"))

    # --- persistent SBUF tensors ---
    # NOTE: S^T deliberately stays ONE big tile (plus st0 for chunk 0). The
    # tile-granular dependency tracking does cause ~12us of false LDWEIGHTS
    # waits behind in-flight chunk DMAs during ib0, but splitting into 64
    # per-chunk tiles was measured 47us WORSE (per-tile semaphore overhead).
    st = singles.tile([128, KT * N], BF16)            # S^T: [k*N + j] layout
    w_sb = singles.tile([128, KT * NOUT], F32R)       # W c-tiles
    cj_sb = singles.tile([128, NJT], F32)             # cj[t*128+p] at [p, t]
    rib_sb = singles.tile([128, R], F32)              # ri/2 broadcast across partitions
    w2b_sb = singles.tile([128, NOUT], F32)           # w2 broadcast across partitions
    ident = singles.tile([128, 128], F32)

    # chunk 0 lives in four standalone tiles (one per k-slice) so the first
    # gram matmul is gated on a single 256 KB DMA, not the whole 1 MB chunk
    st0 = [singles.tile([128, CH], BF16, name=f"st0_{k}") for k in range(KT)]

    def load_st_chunk(c):
        for k in range(KT):
            nc.sync.dma_start(
                out=st[:, k * N + c * CH: k * N + (c + 1) * CH],
                in_=s_d[bass.ts(k, 128), c * CH:(c + 1) * CH],
            )

    def st_slice(k, lo, hi):
        """S^T [k-slice, columns lo:hi]; chunk-0 columns come from st0."""
        if hi <= CH:
            return st0[k][:, lo:hi]
        return st[:, k * N + lo: k * N + hi]

    # chunk 0 first: the first gram matmuls are gated on it; riB isn't needed
    # until the first DVE sub, cj until the first ACT, w2B until the tail
    for k in range(KT):
        nc.sync.dma_start(out=st0[k][:], in_=s_d[bass.ts(k, 128), 0:CH])
    nc.sync.dma_start(out=rib_sb[:], in_=rib_d)
    nc.sync.dma_start(out=cj_sb[:], in_=cj_d)
    nc.sync.dma_start(out=w2b_sb[:], in_=w2b_d)
    make_identity(nc, ident[:])  # only needed at the i-block tails

    def emit_out(ib, s, y0ts, ninv_q):
        """GEMM + scale + w2 + store for one 128-row output sub-tile."""
        ninv, q32 = ninv_q
        pso = ps_o.tile([128, NOUT], F32, tag="o", name=f"pso{ib}_{s}")
        for c in range(KT):
            nc.tensor.matmul(
                pso[:],
                y0ts[c][:, bass.ts(s, 128)],
                w_sb[:, c * NOUT:(c + 1) * NOUT],
                start=(c == 0),
                stop=(c == KT - 1),
            )
        acted = osb_pool.tile([128, NOUT], F32, tag="osb", name=f"acted{ib}_{s}")
        nc.scalar.activation(
            out=acted[:], in_=pso[:],
            func=mybir.ActivationFunctionType.Copy, scale=ninv[:],
        )
        # out = q*w2 + acted  (= w2 - (Yc@W + 32*w2)/rowmax)
        osb = osb_pool.tile([128, NOUT], F32, tag="osb", name=f"osb{ib}_{s}")
        nc.vector.scalar_tensor_tensor(
            out=osb[:], in0=w2b_sb[:], scalar=q32[:], in1=acted[:],
            op0=mybir.AluOpType.mult, op1=mybir.AluOpType.add,
        )
        nc.sync.dma_start(out=out_d[bass.ts(ib * NSUB + s, 128), :], in_=osb[:])

    # --- main: per i-block: gram strip -> G0 -> Y0T accum -> normalize -> GEMM ---
    deferred = []  # output-side tail closures from the previous i-block
    dma_q = []     # pending (chunk, k) background loads, one issued per jt
    for ib in range(NIB):
        icol0 = ib * IB  # local column offset into S^T / riB
        psy = [ps_y.tile([128, IB], F32, tag=f"y{c}", name=f"psy{c}")
               for c in range(KT)]
        macc = macc_pool.tile([128, IB], F32, tag="macc")
        hist = []

        for jt in range(NJT):
            # interleave the previous i-block's output tail into this block's
            # stream so its dependency latency hides under main-loop compute
            if deferred and jt >= 2 and jt % 2 == 0:
                deferred.pop(0)()
            xt = xt_pool.tile([128, D], BF16, tag="xt")
            nc.sync.dma_start(out=xt[:], in_=x_d[bass.ts(jt, 128), :])

            if ib == 0:
                if jt == 0:
                    load_st_chunk(1)
                    load_st_chunk(2)
                elif jt % 4 == 0 and jt // 4 + 2 < NCH:
                    load_st_chunk(jt // 4 + 2)
                if jt == 32:
                    for kt in range(KT):
                        nc.sync.dma_start(
                            out=w_sb[:, kt * NOUT:(kt + 1) * NOUT],
                            in_=w_d[bass.ts(kt, 128), :].bitcast(F32R),
                        )

            psg = ps_g.tile([128, IB], F32, tag="g")
            for k in range(KT):
                nc.tensor.matmul(
                    psg[:],
                    st_slice(k, jt * 128, jt * 128 + 128),
                    st_slice(k, icol0, icol0 + IB),
                    start=(k == 0),
                    stop=(k == KT - 1),
                )
            # sqd = gram - ri/2  (broadcast tile; per-i term of the expansion);
            # PSUM -> SBUF, freeing the psg bank for the next gram group
            sqd = sqd_pool.tile([128, IB], F32, tag="sqd")
            nc.vector.tensor_sub(sqd[:], psg[:], rib_sb[:, icol0:icol0 + IB])

            # G0^T tile = sqrt(-2*sqd + cj[j])   (cj includes +CLAMP)
            g0 = g0_pool.tile([128, IB], F32, tag="g0")
            nc.scalar.activation(
                out=g0[:], in_=sqd[:],
                func=mybir.ActivationFunctionType.Sqrt,
                bias=cj_sb[:, jt:jt + 1], scale=-2.0,
            )
            # centered copy c0 = G0 - 32 in bf16 for the Y matmuls: G0 values
            # cluster near sqrt(2D)=32, so centering keeps the bf16 rounding
            # error ~30x smaller; the 32*colsum component is restored exactly
            # in the tail via w2 (host-computed from the same rounded x).
            # Must stay on the Scalar engine: GpSimd runs this op ~10x slower
            # (microcode overhead) and tanks the whole kernel.
            c0 = c0_pool.tile([128, IB], BF16, tag="c0")
            nc.scalar.activation(
                out=c0[:], in_=g0[:],
                func=mybir.ActivationFunctionType.Copy, bias=-32.0,
            )

            # software pipeline: the Y0T matmuls AND the rowmax update run TWO
            # steps behind the gram, giving the gram->sub->sqrt chain a full
            # extra iteration of slack before Y consumes g0. Emitting sub
            # before max also keeps the psg PSUM recycle off the ACT chain.
            if jt >= 2:
                pg0, pc0, pxt = hist[jt - 2]
                if jt == 2:
                    nc.vector.tensor_copy(out=macc[:], in_=pg0[:])
                else:
                    nc.vector.tensor_max(macc[:], macc[:], pg0[:])
                for c in range(KT):
                    nc.tensor.matmul(
                        psy[c][:], pxt[:, bass.ts(c, 128)], pc0[:],
                        start=(jt == 2), stop=False,
                    )
            hist.append((g0, c0, xt))

        for tail_jt in (NJT - 2, NJT - 1):
            pg0, pc0, pxt = hist[tail_jt]
            nc.vector.tensor_max(macc[:], macc[:], pg0[:])
            for c in range(KT):
                nc.tensor.matmul(
                    psy[c][:], pxt[:, bass.ts(c, 128)], pc0[:],
                    start=False, stop=(tail_jt == NJT - 1),
                )

        # tail, part 1: rowmax -> -1/rowmax per i sub-tile
        ninvs = []
        for s in range(NSUB):
            pst = ps_tr.tile([128, 128], F32, tag="tr")
            nc.tensor.transpose(pst[:], macc[:, bass.ts(s, 128)], ident[:])
            rm = sm_pool.tile([128, 1], F32, tag="rm")
            nc.vector.tensor_reduce(
                out=rm[:], in_=pst[:], axis=mybir.AxisListType.X,
                op=mybir.AluOpType.max,
            )
            nrm = sm_pool.tile([128, 1], F32, tag="nrm")
            nc.vector.tensor_scalar_mul(nrm[:], rm[:], -1.0)
            ninv = sm_pool.tile([128, 1], F32, tag="ninv", name=f"ninv{s}")
            nc.vector.reciprocal(ninv[:], nrm[:])  # -1/rowmax
            # q = 1 - 32/rowmax: scales w2 to restore the centered-away
            # 32*colsum component of Y0 (= G0@x with G0 = c0 + 32)
            q32 = sm_pool.tile([128, 1], F32, tag="q32", name=f"q32{s}")
            nc.vector.tensor_scalar(
                out=q32[:], in0=ninv[:], scalar1=32.0, scalar2=1.0,
                op0=mybir.AluOpType.mult, op1=mybir.AluOpType.add,
            )
            ninvs.append((ninv, q32))

        # tail, part 2: Y0T out of PSUM (frees the psy banks for the next
        # i-block); the GEMM/scale/store closures are deferred into the next
        # i-block's instruction stream (emitted immediately on the last block)
        y0ts = []
        for c in range(KT):
            y0t = y0t_pool.tile([128, IB], F32R, tag="y0t", name=f"y0t{ib}_{c}")
            if c % 2 == 0:
                nc.vector.tensor_copy(out=y0t[:], in_=psy[c][:])
            else:
                nc.scalar.copy(out=y0t[:], in_=psy[c][:])
            y0ts.append(y0t)

        deferred = [
            (lambda ib=ib, s=s, y0ts=y0ts, ninv=ninvs[s]: emit_out(ib, s, y0ts, ninv))
            for s in range(NSUB)
        ]
        if ib == NIB - 1:
            for fn in deferred:
                fn()


_NC_CACHE = {}


def _build_nc():
    if "nc" in _NC_CACHE:
        return _NC_CACHE["nc"]
    nc = bacc.Bacc("TRN2", target_bir_lowering=False, debug=False, num_devices=M)
    x_d = nc.dram_tensor("x", [N, D], BF16, kind="ExternalInput").ap()
    s_d = nc.dram_tensor("simT", [D, N], BF16, kind="ExternalInput").ap()
    cj_d = nc.dram_tensor("cj", [128, NJT], F32, kind="ExternalInput").ap()
    rib_d = nc.dram_tensor("riB", [128, R], F32, kind="ExternalInput").ap()
    w_d = nc.dram_tensor("w", [D, NOUT], F32, kind="ExternalInput").ap()
    w2b_d = nc.dram_tensor("w2B", [128, NOUT], F32, kind="ExternalInput").ap()
    out_d = nc.dram_tensor("out", [R, NOUT], F32, kind="ExternalOutput").ap()
    with tile.TileContext(nc) as tc, ExitStack() as ctx:
        build_kernel(ctx, tc, out_d, x_d, s_d, cj_d, rib_d, w_d, w2b_d)
    nc.compile()
    _NC_CACHE["nc"] = nc
    return nc


def make_in_maps(x, sim_feat, weight):
    w = np.ascontiguousarray(weight, dtype=np.float32)
    # the gram matmuls run in bf16: round sim on the host and derive ri/cj
    # from the ROUNDED values so the device computes exact distances of the
    # rounded vectors (error = distance perturbation only, ~1e-3 relative)
    sim_bf = np.asarray(sim_feat, dtype=np.float32).astype(ml_dtypes.bfloat16)
    # x is also uploaded bf16 (Y stationary). colsum/w2 MUST come from the
    # ROUNDED x: out = w2 - (G0@x_bf@W)/rm cancels the large colsum component
    # exactly only if w2 was built from the same x the device saw; the
    # residual error enters only through G (small), not G0 (large).
    x_bf = np.asarray(x, dtype=np.float32).astype(ml_dtypes.bfloat16)

    sim64 = sim_bf.astype(np.float64)
    sq = (sim64 * sim64).sum(1)
    ss = sim64.sum(1)
    cj_full = (sq - 2.0 * EPS * ss + CLAMP).astype(np.float32)         # [N]
    ri_full = sq + 2.0 * EPS * ss + D * EPS * EPS                      # [N] f64
    colsum = x_bf.astype(np.float64).sum(0)
    w2 = (colsum @ w.astype(np.float64)).astype(np.float32)
    w2B = np.ascontiguousarray(np.broadcast_to(w2, (128, NOUT)))

    in_maps = []
    for c in range(M):
        shift = c * R
        sim_c = np.ascontiguousarray(np.roll(sim_bf, -shift, axis=0).T)
        x_c = np.roll(x_bf, -shift, axis=0)
        cj_c = np.ascontiguousarray(
            np.roll(cj_full, -shift).reshape(NJT, 128).T
        )                                                               # [128, NJT]
        ri_c = (ri_full[shift:shift + R] / 2.0).astype(np.float32)
        rib_c = np.ascontiguousarray(np.broadcast_to(ri_c, (128, R)))
        in_maps.append(
            {"x": x_c, "simT": sim_c, "cj": cj_c, "riB": rib_c,
             "w": w, "w2B": w2B}
        )
    return in_maps


def kernel(x, sim_feat, weight, _trace=False, **kw):
    nc = _build_nc()
    in_maps = make_in_maps(x, sim_feat, weight)
    res = run_bass_kernel_spmd(nc, in_maps, list(range(M)), trace=_trace, **kw)
    out = np.concatenate([res.results[c]["out"] for c in range(M)], axis=0)
    if _trace:
        return out, res
    return out
